# revision 13
# baseline (speedup 1.0000x reference)
"""
2D prefix-max kernel (bottom-pool then right-pool, i.e. cummax over H then W,
output doubled) for x[512, 256, 256] f32, sharded channel-wise over 8 cores.

out = 2 * cummax_w(cummax_h(x))

Per-core plan (64 channels), groups of G=2 channels:
  - Load one [128, (hh, c, w)] = [128, 1024] slab per group (both h-halves).
  - W-cummax: one tensor_tensor_scan along free, segmented per 256 via a bias
    operand (-1e30 at each segment head): state = max(bias + state, x).
  - 8 PE transposes (128x128 fp32) into PSUM T[128, (wh, c, h)].
  - H-cummax: one scan over T (PSUM read), segmented the same way -> S(SBUF).
  - 8 PE transposes back into PSUM Z[128, (hh, c, w)], streaming 2*I as the
    moving operand so Z = 2 * result.
  - ScalarE copy PSUM->SBUF, DMA out.
"""

import numpy as np

from concourse import bacc, bass, mybir
from concourse.bass_utils import run_bass_kernel_spmd
from concourse.masks import make_identity
from concourse.tile import TileContext

C, H, W = 512, 256, 256
N_CORES = 8
C_PER = C // N_CORES  # 64 channels per core
G = 2  # channels per processing group
SEG = G * 2  # 256-long scan segments per tile row
FREE = SEG * W  # 1024
NEG = -1e30

FP32 = mybir.dt.float32

_nc_cache = {}


def _build(c_per: int = C_PER) -> bass.Bass:
    nc = bacc.Bacc()
    x_d = nc.declare_dram_parameter("x", [c_per, H, W], FP32, isOutput=False)
    o_d = nc.declare_dram_parameter("out", [c_per, H, W], FP32, isOutput=True)

    add = mybir.AluOpType.add
    mx = mybir.AluOpType.max

    with TileContext(nc) as tc:
        with (
            tc.tile_pool(name="const", bufs=1) as cpool,
            tc.tile_pool(name="sb", bufs=4) as sb,
            tc.tile_pool(name="ps", bufs=2, space="PSUM") as ps,
        ):
            ident = cpool.tile([128, 128], FP32)
            make_identity(nc, ident[:])

            # Segment-reset bias: -1e30 at free index k*256, 0 elsewhere.
            bias = cpool.tile([128, FREE], FP32)
            nc.vector.memset(bias[:], 0.0)
            for k in range(SEG):
                nc.vector.memset(bias[:, k * W : k * W + 1], NEG)

            for g in range(c_per // G):
                c0 = g * G
                # ---- load both h-halves: X[p=h_local, (hh, c, w)]
                X = sb.tile([128, FREE], FP32, tag="X")
                for u in range(2):
                    nc.sync.dma_start(
                        out=X[:, u * 512 : (u + 1) * 512].rearrange(
                            "p (c w) -> p c w", c=G
                        ),
                        in_=x_d[c0 : c0 + G, u * 128 : (u + 1) * 128, :].rearrange(
                            "c p w -> p c w"
                        ),
                    )
                # ---- W-scan (segments: (hh, c) pairs x 256)
                nc.vector.tensor_tensor_scan(
                    out=X[:], data0=bias[:], data1=X[:], initial=NEG,
                    op0=add, op1=mx,
                )

                # ---- transpose to T[p=w_local, (wh, c, h)]
                T = ps.tile([128, FREE], FP32, tag="T")
                for wh in range(2):
                    for c in range(G):
                        for hh in range(2):
                            o0 = wh * 512 + c * W + hh * 128
                            i0 = hh * 512 + c * W + wh * 128
                            nc.tensor.transpose(
                                T[:, o0 : o0 + 128], X[:, i0 : i0 + 128], ident[:]
                            )
                # ---- H-scan (PSUM read) -> S (SBUF)
                S = sb.tile([128, FREE], FP32, tag="S")
                nc.vector.tensor_tensor_scan(
                    out=S[:], data0=bias[:], data1=T[:], initial=NEG,
                    op0=add, op1=mx,
                )

                # ---- transpose back (x2 via 2*I) to Z[p=h_local, (hh, c, w)]
                Z = ps.tile([128, FREE], FP32, tag="Z")
                for hh in range(2):
                    for c in range(G):
                        for wh in range(2):
                            o0 = hh * 512 + c * W + wh * 128
                            i0 = wh * 512 + c * W + hh * 128
                            nc.tensor.transpose(
                                Z[:, o0 : o0 + 128], S[:, i0 : i0 + 128], ident[:]
                            )
                # ---- PSUM -> SBUF, store
                O = sb.tile([128, FREE], FP32, tag="O")
                nc.scalar.activation(
                    O[:], Z[:], mybir.ActivationFunctionType.Copy, scale=2.0
                )
                for u in range(2):
                    nc.sync.dma_start(
                        out=o_d[
                            c0 : c0 + G, u * 128 : (u + 1) * 128, :
                        ].rearrange("c p w -> p c w"),
                        in_=O[:, u * 512 : (u + 1) * 512].rearrange(
                            "p (c w) -> p c w", c=G
                        ),
                    )
    nc.finalize()
    return nc


def run(x: np.ndarray, trace: bool = False):
    """Run on 8 cores; returns (full_output, BassKernelResults)."""
    if "full" not in _nc_cache:
        _nc_cache["full"] = _build()
    nc = _nc_cache["full"]
    in_maps = [
        {"x": np.ascontiguousarray(x[i * C_PER : (i + 1) * C_PER])}
        for i in range(N_CORES)
    ]
    res = run_bass_kernel_spmd(nc, in_maps, list(range(N_CORES)), trace=trace)
    out = np.concatenate([res.results[i]["out"] for i in range(N_CORES)], axis=0)
    return out, res


def kernel(x: np.ndarray) -> np.ndarray:
    out, _ = run(np.asarray(x), trace=False)
    return out


# revision 18
# speedup vs baseline: 1.1074x; 1.1074x over previous
"""
2D prefix-max kernel (bottom-pool then right-pool, i.e. cummax over H then W,
output doubled) for x[512, 256, 256] f32, sharded channel-wise over 8 cores.

out = 2 * cummax_w(cummax_h(x))

Per-core plan (64 channels), groups of G=2 channels:
  - Load one [128, (hh, c, w)] = [128, 1024] slab per group (both h-halves).
  - W-cummax: one tensor_tensor_scan along free, segmented per 256 via a bias
    operand (-1e30 at each segment head): state = max(bias + state, x).
  - 8 PE transposes (128x128 fp32) into PSUM T[128, (wh, c, h)].
  - H-cummax: one scan over T (PSUM read), segmented the same way -> S(SBUF).
  - 8 PE transposes back into PSUM Z[128, (hh, c, w)], streaming 2*I as the
    moving operand so Z = 2 * result.
  - ScalarE copy PSUM->SBUF, DMA out.
"""

import numpy as np

from concourse import bacc, bass, mybir
from concourse.bass_utils import run_bass_kernel_spmd
from concourse.masks import make_identity
from concourse.tile import TileContext

C, H, W = 512, 256, 256
N_CORES = 8
C_PER = C // N_CORES  # 64 channels per core
G = 2  # channels per processing group
SEG = G * 2  # 256-long scan segments per tile row
FREE = SEG * W  # 1024
NEG = -1e30

FP32 = mybir.dt.float32
FP32R = mybir.dt.float32r

_nc_cache = {}


def _build(c_per: int = C_PER) -> bass.Bass:
    nc = bacc.Bacc()
    x_d = nc.declare_dram_parameter("x", [c_per, H, W], FP32, isOutput=False)
    o_d = nc.declare_dram_parameter("out", [c_per, H, W], FP32, isOutput=True)

    add = mybir.AluOpType.add
    mx = mybir.AluOpType.max

    with TileContext(nc) as tc:
        with (
            tc.tile_pool(name="const", bufs=1) as cpool,
            tc.tile_pool(name="sb", bufs=4) as sb,
            tc.tile_pool(name="ps", bufs=2, space="PSUM") as ps,
        ):
            ident = cpool.tile([128, 128], FP32)
            make_identity(nc, ident[:])

            # Segment-reset bias: -1e30 at free index k*256, 0 elsewhere.
            bias = cpool.tile([128, FREE], FP32)
            nc.vector.memset(bias[:], 0.0)
            for k in range(SEG):
                nc.vector.memset(bias[:, k * W : k * W + 1], NEG)

            n_groups = c_per // G
            # Software-pipelined with a 1-group skew so DVE's W-scan(g)
            # overlaps PE's transposes of group g-1 (keeps PE dense -> HAM
            # ramp; avoids DVE<->PE ping-pong serialization).
            Ts, Ss = {}, {}

            def stage_front(g):
                c0 = g * G
                X = sb.tile([128, FREE], FP32, tag="X")
                for u in range(2):
                    nc.sync.dma_start(
                        out=X[:, u * 512 : (u + 1) * 512].rearrange(
                            "p (c w) -> p c w", c=G
                        ),
                        in_=x_d[c0 : c0 + G, u * 128 : (u + 1) * 128, :].rearrange(
                            "c p w -> p c w"
                        ),
                    )
                # W-scan (segments: (hh, c) pairs x 256)
                nc.vector.tensor_tensor_scan(
                    out=X[:], data0=bias[:], data1=X[:], initial=NEG,
                    op0=add, op1=mx,
                )
                # transpose to T[p=w_local, (wh, c, h)]
                T = ps.tile([128, FREE], FP32, tag="T")
                for wh in range(2):
                    for c in range(G):
                        for hh in range(2):
                            o0 = wh * 512 + c * W + hh * 128
                            i0 = hh * 512 + c * W + wh * 128
                            nc.tensor.transpose(
                                T[:, o0 : o0 + 128], X[:, i0 : i0 + 128], ident[:]
                            )
                Ts[g] = T

            def stage_mid(g):
                # H-scan (PSUM read) -> S (SBUF)
                S = sb.tile([128, FREE], FP32, tag="S")
                nc.vector.tensor_tensor_scan(
                    out=S[:], data0=bias[:], data1=Ts.pop(g)[:], initial=NEG,
                    op0=add, op1=mx,
                )
                # transpose back to Z[p=h_local, (hh, c, w)]
                Z = ps.tile([128, FREE], FP32, tag="Z")
                for hh in range(2):
                    for c in range(G):
                        for wh in range(2):
                            o0 = hh * 512 + c * W + wh * 128
                            i0 = wh * 512 + c * W + hh * 128
                            nc.tensor.transpose(
                                Z[:, o0 : o0 + 128], S[:, i0 : i0 + 128], ident[:]
                            )
                c0 = g * G
                O = sb.tile([128, FREE], FP32, tag="O")
                nc.scalar.activation(
                    O[:], Z[:], mybir.ActivationFunctionType.Copy, scale=2.0
                )
                for u in range(2):
                    nc.sync.dma_start(
                        out=o_d[
                            c0 : c0 + G, u * 128 : (u + 1) * 128, :
                        ].rearrange("c p w -> p c w"),
                        in_=O[:, u * 512 : (u + 1) * 512].rearrange(
                            "p (c w) -> p c w", c=G
                        ),
                    )

            for g in range(n_groups + 1):
                if g < n_groups:
                    stage_front(g)
                if g >= 1:
                    stage_mid(g - 1)
    nc.finalize()
    return nc


def run(x: np.ndarray, trace: bool = False):
    """Run on 8 cores; returns (full_output, BassKernelResults)."""
    if "full" not in _nc_cache:
        _nc_cache["full"] = _build()
    nc = _nc_cache["full"]
    in_maps = [
        {"x": np.ascontiguousarray(x[i * C_PER : (i + 1) * C_PER])}
        for i in range(N_CORES)
    ]
    res = run_bass_kernel_spmd(nc, in_maps, list(range(N_CORES)), trace=trace)
    out = np.concatenate([res.results[i]["out"] for i in range(N_CORES)], axis=0)
    return out, res


def kernel(x: np.ndarray) -> np.ndarray:
    out, _ = run(np.asarray(x), trace=False)
    return out
